# revision 1
# baseline (speedup 1.0000x reference)
"""Two-layer LSTM (H=51) over [B=4096, T=256] on 8 NeuronCores.

Strategy: data-parallel over batch (512 per core). Per core, a skewed
software pipeline over T+2 phases: phase q computes layer-1 of step q,
layer-2 of step q-1, and the linear head of step q-2.

All sigmoids are rewritten as tanh (sigma(z) = (tanh(z/2)+1)/2) with the
1/2 folded into host-precomputed weights, and states stored doubled
(ht = 2h, ct = 2c), so each phase needs only two ACT instructions
(one tanh over all eight gate matmul outputs, one tanh(c/2)) and four
fused scalar_tensor_tensor DVE instructions.
"""

import numpy as np

H = 51
T_FULL = 256
B_FULL = 4096
N_CORES = 8

# Stk partition layout (stacked matmul rhs). Compute-engine writes must
# start at a 32-aligned partition, so the states live at rows 0..114
# (matching the gate-row space) and the constant rows sit above them:
#   rows 0..50   : ht1 (= 2*h1)
#   rows 51..63  : junk (zero, weighted by zero)
#   rows 64..114 : ht2 (= 2*h2)
#   row 115      : ones (bias row, DMA-initialized)
#   row 116      : x_t (DMA per step)
ROW_H1 = 0
ROW_JUNK = 51
ROW_H2 = 64
ROW_ONES = 115
ROW_X = 116
K_STK = 117
# gate-row space of the elementwise ops: rows 0..50 layer1, 51..63 junk,
# 64..114 layer2
GP = 115


MW = GP  # matmul output width (zero-padded gate lhsT columns)


def _build_weights(W_ih1, W_hh1, b_ih1, b_hh1, W_ih2, W_hh2, b_ih2, b_hh2,
                   W_lin, b_lin):
    """Host-side packing of lhsT weight tiles.

    Returns WG [K_STK, 8*MW + 1] float32. Eight gate lhsTs of width MW=115
    (layer1 i,f,o,g then layer2 i,f,o,g), zero-padded so that both layers'
    matmuls write the full [115, B] PSUM region at partition base 0 (f32r
    matmuls require base 0): layer-1 weights occupy output rows 0..50 and
    clear the rest with zero columns (start=True); layer-2 weights occupy
    rows 64..114 and accumulate (start=False).
    Column 8*MW rows 64..115: [0.5*W_lin; b_lin] for the out head
    (lhsT partitions must match its rhs Stk[64:116] = [ht2; ones]).
    Gate scaling: sigma-gates (i,f,o) rows scaled by 0.5 (tanh(z/2) trick);
    h inputs scaled by 0.5 (states stored doubled).
    """
    b1 = (b_ih1 + b_hh1).astype(np.float64)
    b2 = (b_ih2 + b_hh2).astype(np.float64)
    # reference gate order in the stacked 4H rows: i, f, g, o
    idx = {"i": np.arange(0, H), "f": np.arange(H, 2 * H),
           "g": np.arange(2 * H, 3 * H), "o": np.arange(3 * H, 4 * H)}
    # our bank order: i, g first (v = (ti+1)*tg depends only on these, so
    # the first of the two split tanh ops unblocks it), then f, o
    order = ["i", "g", "f", "o"]
    WG = np.zeros((K_STK, 8 * MW + 1), dtype=np.float64)
    for xi, gate in enumerate(order):
        r = idx[gate]
        s = 0.5 if gate in ("i", "f", "o") else 1.0
        col = slice(xi * MW, xi * MW + H)  # output rows 0..50 (noqa)
        # layer 1: z1 = W_ih1 @ x + b1 + W_hh1 @ h1
        WG[ROW_ONES, col] = s * b1[r]
        WG[ROW_H1:ROW_H1 + H, col] = s * 0.5 * W_hh1[r, :].T
        WG[ROW_X, col] = s * W_ih1[r, 0]
    for xi, gate in enumerate(order):
        r = idx[gate]
        s = 0.5 if gate in ("i", "f", "o") else 1.0
        col = slice((4 + xi) * MW + ROW_H2, (4 + xi) * MW + ROW_H2 + H)
        # layer 2: z2 = W_ih2 @ h1 + b2 + W_hh2 @ h2 (output rows 64..114)
        WG[ROW_ONES, col] = s * b2[r]
        WG[ROW_H1:ROW_H1 + H, col] = s * 0.5 * W_ih2[r, :].T
        WG[ROW_H2:ROW_H2 + H, col] = s * 0.5 * W_hh2[r, :].T
    # out head: lhsT must sit at the same partitions as its rhs Stk[64:116]
    # (= [ht2 (51); ones]), so W_lin goes at rows 64..114 and b_lin at 115.
    WG[ROW_H2:ROW_H2 + H, 8 * MW] = 0.5 * W_lin[0, :]
    WG[ROW_ONES, 8 * MW] = float(np.asarray(b_lin).reshape(-1)[0])
    return np.ascontiguousarray(WG).astype(np.float32)


def build_core_kernel(T, B, groups=2, use_f32r=True):
    """Build the per-core Bass kernel. Inputs: xT [T, B], WG [K_STK, 409].
    Output: outT [T, B] (linear head, WITHOUT b_lin)."""
    import concourse.bacc as bacc
    import concourse.mybir as mybir
    from concourse.tile import TileContext

    fp = mybir.dt.float32
    # Matmul operands are float32r (full-rate fp32 path). The verifier
    # requires f32r typing end-to-end, so the state/weight/x tensors are
    # natively f32r; h-tilde is only ever consumed by matmuls, so rounding
    # at the DVE write loses nothing vs rounding at the PE read.
    fpr = mybir.dt.float32r if use_f32r else fp
    Bg = B // groups

    nc = bacc.Bacc("TRN2", target_bir_lowering=False, debug=False)
    # xT row 0 is a host-prepended row of ones (feeds the bias row of Stk);
    # rows 1..T are input.T
    xT = nc.dram_tensor("xT", [T + 1, B], fpr, kind="ExternalInput")
    WG = nc.dram_tensor("WG", [K_STK, 8 * MW + 1], fpr, kind="ExternalInput")
    out_bt = nc.dram_tensor("out_bt", [B, T], fp, kind="ExternalOutput")

    C = min(128, T)  # output columns buffered in PSUM between flushes
    assert T % C == 0
    assert (B // groups) % 128 == 0, "batch per group must be a multiple of 128"

    with TileContext(nc) as tc:
        with (
            tc.tile_pool(name="persist", bufs=1) as persist,
            tc.tile_pool(name="gpsum", bufs=1, space="PSUM") as gpsum,
            tc.tile_pool(name="opsum", bufs=1, space="PSUM") as opsum,
            tc.tile_pool(name="temps", bufs=3) as temps,
            tc.tile_pool(name="ostage", bufs=2) as ostage,
        ):
            wg = persist.tile([K_STK, 8 * MW + 1], fpr)
            nc.sync.dma_start(out=wg, in_=WG[:, :])

            nchunk = Bg // 128
            stks, cts, gps, pos = [], [], [], []
            for g in range(groups):
                stk = persist.tile([K_STK, Bg], fpr, tag=f"stk{g}")
                ct = persist.tile([GP, Bg], fp, tag=f"ct{g}")
                gp = gpsum.tile([GP, 4 * Bg], fp, tag=f"gp{g}")
                # DVE memset can't target f32r directly; write zero bits
                # through an f32 view (0.0 is exact in f32r).
                nc.vector.memset(stk[:, :].bitcast(fp), 0.0)
                nc.sync.dma_start(out=stk[ROW_ONES:ROW_ONES + 1, :],
                                  in_=xT[0:1, g * Bg:(g + 1) * Bg])
                nc.vector.memset(ct[:, :], 0.0)
                stks.append(stk)
                cts.append(ct)
                gps.append(gp)
                pos.append(opsum.tile([128, nchunk * C], fp, tag=f"po{g}",
                                      name=f"po{g}"))

            add = mybir.AluOpType.add
            mult = mybir.AluOpType.mult
            tanh = mybir.ActivationFunctionType.Tanh

            for q in range(T + 2):
                for g in range(groups):
                    stk, ct, gp = stks[g], cts[g], gps[g]
                    cols = slice(g * Bg, (g + 1) * Bg)
                    # ---- x load for step q (xT is offset by the ones row)
                    if q < T:
                        nc.sync.dma_start(out=stk[ROW_X:ROW_X + 1, :],
                                          in_=xT[q + 1:q + 2, cols])
                    # ---- gate matmuls: both layers write the full [115, Bg]
                    # region at base 0 (f32r needs base 0); layer-1's
                    # zero-padded lhsT clears rows 51..114, layer-2
                    # accumulates into rows 64..114.
                    rhs = stk[0:K_STK, :]
                    l1 = q < T
                    l2 = 1 <= q <= T
                    for xi in range(4):
                        if l1:
                            nc.tensor.matmul(
                                gp[0:GP, xi * Bg:(xi + 1) * Bg],
                                wg[0:K_STK, xi * MW:xi * MW + MW],
                                rhs, start=True, stop=not l2)
                        if l2:
                            nc.tensor.matmul(
                                gp[0:GP, xi * Bg:(xi + 1) * Bg],
                                wg[0:K_STK, (4 + xi) * MW:(5 + xi) * MW],
                                rhs, start=not l1, stop=True)
                    # ---- out head for step t = q-2: out[:, t] column
                    if q >= 2:
                        t = q - 2
                        tc_col = t % C
                        for k in range(nchunk):
                            # f32r rejects N=1 matmuls; run the tiny out
                            # head in plain f32 via bitcast views.
                            nc.tensor.matmul(
                                pos[g][:, k * C + tc_col:k * C + tc_col + 1],
                                stk[64:116, k * 128:(k + 1) * 128].bitcast(fp),
                                wg[64:116, 8 * MW:8 * MW + 1].bitcast(fp),
                                start=True, stop=True)
                        if tc_col == C - 1:  # flush epoch
                            t0 = t - (C - 1)
                            for k in range(nchunk):
                                st = ostage.tile([128, C], fp, tag=f"os{g}_{k}")
                                nc.scalar.copy(st, pos[g][:, k * C:(k + 1) * C])
                                row0 = g * Bg + k * 128
                                nc.sync.dma_start(
                                    out=out_bt[row0:row0 + 128, t0:t0 + C],
                                    in_=st)
                    # ---- elementwise chain (banks: 0=i, 1=g, 2=f, 3=o).
                    # tanh is split in two so v = (ti+1)*tg can start after
                    # only the i,g matmuls; f,o matmuls overlap the first
                    # tanh on the PE.
                    if q <= T:
                        tg_t = temps.tile([GP, 4 * Bg], fp, tag=f"tg{g}")
                        nc.scalar.activation(tg_t[:, 0:2 * Bg],
                                             gp[0:GP, 0:2 * Bg], tanh)
                        nc.scalar.activation(tg_t[:, 2 * Bg:4 * Bg],
                                             gp[0:GP, 2 * Bg:4 * Bg], tanh)
                        ti = tg_t[:, 0 * Bg:1 * Bg]
                        tg = tg_t[:, 1 * Bg:2 * Bg]
                        tf = tg_t[:, 2 * Bg:3 * Bg]
                        to = tg_t[:, 3 * Bg:4 * Bg]
                        u = temps.tile([GP, Bg], fp, tag=f"u{g}")
                        v = temps.tile([GP, Bg], fp, tag=f"v{g}")
                        tcl = temps.tile([GP, Bg], fp, tag=f"tc{g}")
                        # v = (ti+1)*tg ; u = (tf+1)*ct ; ct = 0.5*u + v
                        nc.vector.scalar_tensor_tensor(v, ti, 1.0, tg, add, mult)
                        nc.vector.scalar_tensor_tensor(u, tf, 1.0, ct[:, :], add, mult)
                        nc.vector.scalar_tensor_tensor(ct[:, :], u, 0.5, v, mult, add)
                        # tanh(c) = tanh(0.5*ct); ht = (to+1)*tanh(c)
                        nc.scalar.activation(tcl, ct[:, :], tanh, scale=0.5)
                        nc.vector.scalar_tensor_tensor(
                            stk[ROW_H1:ROW_H1 + GP, :], to, 1.0, tcl, add, mult)
    nc.compile()
    return nc


_NC_CACHE = {}


def _get_nc(T, B, groups, use_f32r):
    key = (T, B, groups, use_f32r)
    if key not in _NC_CACHE:
        _NC_CACHE[key] = build_core_kernel(T, B, groups, use_f32r)
    return _NC_CACHE[key]


def kernel(input, W_ih1, W_hh1, b_ih1, b_hh1, W_ih2, W_hh2, b_ih2, b_hh2,
           W_lin, b_lin, _groups=2, _use_f32r=True):
    from concourse import bass_utils

    input = np.asarray(input, dtype=np.float32)
    B, T = input.shape
    Bc = B // N_CORES
    WG = _build_weights(np.asarray(W_ih1, np.float64), np.asarray(W_hh1, np.float64),
                        np.asarray(b_ih1, np.float64), np.asarray(b_hh1, np.float64),
                        np.asarray(W_ih2, np.float64), np.asarray(W_hh2, np.float64),
                        np.asarray(b_ih2, np.float64), np.asarray(b_hh2, np.float64),
                        np.asarray(W_lin, np.float64), np.asarray(b_lin, np.float64))
    # row 0 = ones (bias row), rows 1..T = input.T
    xT = np.concatenate([np.ones((1, B), np.float32), input.T.astype(np.float32)])
    nc = _get_nc(T, Bc, _groups, _use_f32r)
    in_maps = [
        {"xT": np.ascontiguousarray(xT[:, c * Bc:(c + 1) * Bc]), "WG": WG}
        for c in range(N_CORES)
    ]
    res = bass_utils.run_bass_kernel_spmd(
        nc, in_maps, core_ids=list(range(N_CORES)), trace=False)
    outs = [res.results[c]["out_bt"] for c in range(N_CORES)]  # [Bc, T] each
    out = np.concatenate(outs, axis=0)  # [B, T]
    return out.astype(np.float32)



# revision 2
# speedup vs baseline: 1.0011x; 1.0011x over previous
"""Two-layer LSTM (H=51) over [B=4096, T=256] on 8 NeuronCores — V3.

V3 = V2 (merged L1+L2 matmuls, ping-pong stk, prefetched x) plus:
  * Direct Sigmoid activation for i,f,o gates (Tanh only for g and c).
    Both live in the `sigmoid_and_others` table set -> no per-step table
    reloads. This removes the (t+1) adjustments so every elementwise op
    becomes a plain tensor_tensor, which the DVE runs at 2x in bf16
    (194ns vs 327ns for scalar_tensor_tensor at [115,256]).
  * bf16 states/weights/activations end-to-end (PSUM accumulation stays
    fp32; head outputs stay fp32). bf16 matmuls run 1 cycle/row at any
    size (no f32r N>=256 constraint).

Gate bank layout in PSUM free dim: [i | f | o | g].
act_layout:
  "Y": sigma(ifo) [115,3Bg] + tanh(g) [115,Bg] + tanh(c)   (3 ACT instrs)
  "X": sigma(if) + tanh(g) + sigma(o) + tanh(c)            (4 ACT instrs,
       shorter critical path: u=sigma_f*c starts after sigma(if) only)
"""

import numpy as np

H = 51
T_FULL = 256
B_FULL = 4096
N_CORES = 8

ROW_H1 = 0
ROW_H2 = 64
ROW_ONES = 115
ROW_X = 116
K_STK = 117
GP = 115
MW = GP


def _build_weights(W_ih1, W_hh1, b_ih1, b_hh1, W_ih2, W_hh2, b_ih2, b_hh2,
                   W_lin, b_lin, order=("i", "f", "o", "g"),
                   row_scale=None, h_scale=1.0, head_scale=1.0):
    """WG [K_STK, 12*MW + 1] float32 (cast to bf16 by caller):
    blocks 0..3 combined (banks in `order`), 4..7 L1-only, 8..11 L2-only,
    col 12*MW head [head_scale*W_lin; b_lin] at rows 64..115.
    row_scale[gate]: scale on that gate's rows (tanh(z/2) trick);
    h_scale: scale on h-input columns (doubled-h states)."""
    b1 = (b_ih1 + b_hh1).astype(np.float64)
    b2 = (b_ih2 + b_hh2).astype(np.float64)
    idx = {"i": np.arange(0, H), "f": np.arange(H, 2 * H),
           "g": np.arange(2 * H, 3 * H), "o": np.arange(3 * H, 4 * H)}
    rs = row_scale or {}
    WG = np.zeros((K_STK, 12 * MW + 1), dtype=np.float64)
    for xi, gate in enumerate(order):
        r = idx[gate]
        s = rs.get(gate, 1.0)
        for blk in (xi, 4 + xi):
            col = slice(blk * MW, blk * MW + H)
            WG[ROW_ONES, col] = s * b1[r]
            WG[ROW_H1:ROW_H1 + H, col] = s * h_scale * W_hh1[r, :].T
            WG[ROW_X, col] = s * W_ih1[r, 0]
        for blk in (xi, 8 + xi):
            col = slice(blk * MW + ROW_H2, blk * MW + ROW_H2 + H)
            WG[ROW_ONES, col] = s * b2[r]
            WG[ROW_H1:ROW_H1 + H, col] = s * h_scale * W_ih2[r, :].T
            WG[ROW_H2:ROW_H2 + H, col] = s * h_scale * W_hh2[r, :].T
    WG[ROW_H2:ROW_H2 + H, 12 * MW] = head_scale * W_lin[0, :]
    WG[ROW_ONES, 12 * MW] = float(np.asarray(b_lin).reshape(-1)[0])
    return np.ascontiguousarray(WG).astype(np.float32)


def build_core_kernel(T, B, groups=2, act_layout="X", dt_c="bf16",
                      mm_perm=None, u_late=False, stagger=0):
    if act_layout == "Z":
        return build_core_kernel_z(T, B, dt_c)
    if act_layout == "XO":
        return build_core_kernel_xo(T, B, dt_c)
    import concourse.bacc as bacc
    import concourse.mybir as mybir
    from concourse.tile import TileContext

    fp = mybir.dt.float32
    bf = mybir.dt.bfloat16
    fc = bf if dt_c == "bf16" else fp
    Bg = B // groups

    nc = bacc.Bacc("TRN2", target_bir_lowering=False, debug=False)
    xT = nc.dram_tensor("xT", [T + 1, B], bf, kind="ExternalInput")
    WG = nc.dram_tensor("WG", [K_STK, 12 * MW + 1], bf, kind="ExternalInput")
    out_bt = nc.dram_tensor("out_bt", [B, T], fp, kind="ExternalOutput")

    C = min(128, T)
    assert T % C == 0
    assert Bg % 128 == 0

    with TileContext(nc) as tc:
        with (
            tc.tile_pool(name="persist", bufs=1) as persist,
            tc.tile_pool(name="gpsum", bufs=1, space="PSUM") as gpsum,
            tc.tile_pool(name="opsum", bufs=1, space="PSUM") as opsum,
            tc.tile_pool(name="temps", bufs=3) as temps,
            tc.tile_pool(name="ostage", bufs=2) as ostage,
        ):
            wg = persist.tile([K_STK, 12 * MW + 1], bf)
            nc.sync.dma_start(out=wg, in_=WG[:, :])

            nchunk = Bg // 128
            forced = act_layout in ("XF", "WF")
            zbs = []
            if forced:
                for g in range(groups):
                    zb = persist.tile([GP, 1], fp, tag=f"zb{g}", name=f"zb{g}")
                    nc.vector.memset(zb[:, :], 0.0)
                    zbs.append(zb)
            stks, cts, gps, pos = [], [], [], []
            for g in range(groups):
                pair = []
                for par in range(2):
                    stk = persist.tile([K_STK, Bg], bf, tag=f"stk{g}_{par}",
                                       name=f"stk{g}_{par}")
                    nc.vector.memset(stk[:, :], 0.0)
                    nc.sync.dma_start(out=stk[ROW_ONES:ROW_ONES + 1, :],
                                      in_=xT[0:1, g * Bg:(g + 1) * Bg])
                    pair.append(stk)
                ct = persist.tile([GP, Bg], fc, tag=f"ct{g}", name=f"ct{g}")
                nc.vector.memset(ct[:, :], 0.0)
                gp = gpsum.tile([GP, 4 * Bg], fp, tag=f"gp{g}", name=f"gp{g}")
                stks.append(pair)
                cts.append(ct)
                gps.append(gp)
                pos.append(opsum.tile([128, nchunk * C], fp, tag=f"po{g}",
                                      name=f"po{g}"))
            for g in range(groups):
                cols = slice(g * Bg, (g + 1) * Bg)
                for par in range(2):
                    nc.sync.dma_start(out=stks[g][par][ROW_X:ROW_X + 1, :],
                                      in_=xT[par + 1:par + 2, cols])
            if stagger:
                # serial DVE dummy chain delaying group 1's first matmul,
                # to tune the inter-group phase offset the schedule locks
                # into (writes an exact 0.0 over a zeroed stk element)
                dly = persist.tile([1, 4], fp, tag="dly", name="dly")
                nc.vector.memset(dly[:, :], 0.0)
                for _ in range(stagger):
                    nc.vector.tensor_scalar_add(dly[:, :], dly[:, :], 0.0)
                for par in range(2):
                    nc.vector.tensor_scalar_mul(
                        stks[1][par].bitcast(fp)[0:1, 0:1],
                        dly[:, 0:1], 0.0)

            add = mybir.AluOpType.add
            mult = mybir.AluOpType.mult
            tanh = mybir.ActivationFunctionType.Tanh
            sigm = mybir.ActivationFunctionType.Sigmoid

            # bank order in free dim: 0=i, 1=f, 2=o, 3=g
            # matmul emission order: i, f first (unblocks sigma(if)),
            # then g (for X's tanh-g) or o (for Y's sigma-ifo)
            if act_layout == "V":
                # chunks: 0=f, 1=g, 2=i, 3=o ; tanh[f g], sigma[i o]
                mm_order = [0, 1, 2, 3]
            elif act_layout in ("X", "X2", "XF"):
                mm_order = [0, 1, 3, 2]
            elif act_layout == "Yg":
                mm_order = [3, 0, 1, 2]
            elif act_layout in ("W", "W2", "WF"):
                # banks: 0=i, 1=f, 2=g, 3=o ; sigma(if), tanh(go)
                mm_order = [0, 1, 2, 3]
            else:
                mm_order = [0, 1, 2, 3]
            if mm_perm is not None:
                mm_order = list(mm_perm)

            for q in range(T + 2):
                for g in range(groups):
                    ct, gp = cts[g], gps[g]
                    stk = stks[g][q % 2]
                    stk_n = stks[g][(q + 1) % 2]
                    cols = slice(g * Bg, (g + 1) * Bg)
                    if q + 2 < T:
                        nc.sync.dma_start(out=stk[ROW_X:ROW_X + 1, :],
                                          in_=xT[q + 3:q + 4, cols])
                    rhs = stk[0:K_STK, :]
                    if q <= T:
                        blk0 = 4 if q == 0 else (8 if q == T else 0)
                        for xi in mm_order:
                            b = blk0 + xi
                            nc.tensor.matmul(
                                gp[0:GP, xi * Bg:(xi + 1) * Bg],
                                wg[0:K_STK, b * MW:(b + 1) * MW],
                                rhs, start=True, stop=True)
                    if q >= 2:
                        t = q - 2
                        tc_col = t % C
                        for k in range(nchunk):
                            nc.tensor.matmul(
                                pos[g][:, k * C + tc_col:k * C + tc_col + 1],
                                stk[64:116, k * 128:(k + 1) * 128],
                                wg[64:116, 12 * MW:12 * MW + 1],
                                start=True, stop=True)
                        if tc_col == C - 1:
                            t0 = t - (C - 1)
                            for k in range(nchunk):
                                st = ostage.tile([128, C], fp, tag=f"os{g}_{k}",
                                                 name=f"os{g}_{k}")
                                nc.vector.tensor_scalar_add(
                                    st, pos[g][:, k * C:(k + 1) * C], 0.0)
                                row0 = g * Bg + k * 128
                                nc.sync.dma_start(
                                    out=out_bt[row0:row0 + 128, t0:t0 + C],
                                    in_=st)
                    if q <= T:
                        tg_t = temps.tile([GP, 4 * Bg], bf, tag=f"tg{g}",
                                          name=f"tg{g}")
                        si = tg_t[:, 0 * Bg:1 * Bg]
                        sf = tg_t[:, 1 * Bg:2 * Bg]
                        so = tg_t[:, 2 * Bg:3 * Bg]
                        tg = tg_t[:, 3 * Bg:4 * Bg]
                        u = temps.tile([GP, Bg], fc, tag=f"u{g}", name=f"u{g}")
                        v = temps.tile([GP, Bg], fc, tag=f"v{g}", name=f"v{g}")
                        tcl = temps.tile([GP, Bg], bf, tag=f"tc{g}",
                                         name=f"tc{g}")
                        if act_layout == "V":
                            # chunks: 0=f, 1=g, 2=i, 3=o. f in (tanh+1)/2
                            # form (rows pre-scaled 0.5); c,h stored plain.
                            tf_v = tg_t[:, 0 * Bg:1 * Bg]
                            tg_v = tg_t[:, 1 * Bg:2 * Bg]
                            si_v = tg_t[:, 2 * Bg:3 * Bg]
                            so_v = tg_t[:, 3 * Bg:4 * Bg]
                            nc.scalar.activation(tg_t[:, 0:2 * Bg],
                                                 gp[0:GP, 0:2 * Bg], tanh)
                            # u = (tf+1)*c
                            nc.vector.scalar_tensor_tensor(u, tf_v, 1.0,
                                                           ct[:, :], add, mult)
                            nc.scalar.activation(tg_t[:, 2 * Bg:4 * Bg],
                                                 gp[0:GP, 2 * Bg:4 * Bg], sigm)
                            # v = sigma_i * tg ; c' = 0.5u + v
                            nc.vector.tensor_tensor(v, si_v, tg_v, mult)
                            nc.vector.scalar_tensor_tensor(ct[:, :], u, 0.5,
                                                           v, mult, add)
                            nc.scalar.activation(tcl, ct[:, :], tanh)
                            nc.vector.tensor_tensor(
                                stk_n[ROW_H1:ROW_H1 + GP, :], so_v, tcl, mult)
                        elif act_layout in ("W", "W2", "WF"):
                            # banks in free dim: 0=i, 1=f, 2=g, 3=o
                            # h stored doubled: ht=(to+1)*tcl; c plain.
                            sif = tg_t[:, 0:2 * Bg]
                            si_w = tg_t[:, 0 * Bg:1 * Bg]
                            sf_w = tg_t[:, 1 * Bg:2 * Bg]
                            tg_w = tg_t[:, 2 * Bg:3 * Bg]
                            to_w = tg_t[:, 3 * Bg:4 * Bg]
                            bias = zbs[1 - g][:, 0:1] if forced else 0.0
                            nc.scalar.activation(sif, gp[0:GP, 0:2 * Bg], sigm,
                                                 bias=bias)
                            nc.vector.tensor_tensor(u, sf_w, ct[:, :], mult)
                            nc.scalar.activation(tg_t[:, 2 * Bg:4 * Bg],
                                                 gp[0:GP, 2 * Bg:4 * Bg], tanh)
                            nc.vector.tensor_tensor(v, si_w, tg_w, mult)
                            nc.vector.tensor_tensor(ct[:, :], u, v, add)
                            if forced:
                                nc.vector.tensor_scalar_mul(zbs[g][:, :],
                                                            ct[:, 0:1], 0.0)
                            nc.scalar.activation(tcl, ct[:, :], tanh)
                            nc.vector.scalar_tensor_tensor(
                                stk_n[ROW_H1:ROW_H1 + GP, :], to_w, 1.0, tcl,
                                add, mult)
                        elif act_layout == "X2":
                            # X with sigma(o) emitted BEFORE tanh-c so the
                            # unsatisfied tanh-c doesn't clog the 4-deep ACT
                            # wait queue ahead of ready instructions
                            nc.scalar.activation(tg_t[:, 0:2 * Bg],
                                                 gp[0:GP, 0:2 * Bg], sigm)
                            nc.vector.tensor_tensor(u, sf, ct[:, :], mult)
                            nc.scalar.activation(tg, gp[0:GP, 3 * Bg:4 * Bg],
                                                 tanh)
                            nc.scalar.activation(so, gp[0:GP, 2 * Bg:3 * Bg],
                                                 sigm)
                            nc.vector.tensor_tensor(v, si, tg, mult)
                            nc.vector.tensor_tensor(ct[:, :], u, v, add)
                            nc.scalar.activation(tcl, ct[:, :], tanh)
                            nc.vector.tensor_tensor(
                                stk_n[ROW_H1:ROW_H1 + GP, :], so, tcl, mult)
                        elif act_layout in ("X", "XF"):
                            # sigma(if) -> u early; tanh(g) -> v; sigma(o) late
                            bias = zbs[1 - g][:, 0:1] if forced else 0.0
                            nc.scalar.activation(tg_t[:, 0:2 * Bg],
                                                 gp[0:GP, 0:2 * Bg], sigm,
                                                 bias=bias)
                            if not u_late:
                                nc.vector.tensor_tensor(u, sf, ct[:, :], mult)
                            nc.scalar.activation(tg, gp[0:GP, 3 * Bg:4 * Bg],
                                                 tanh)
                            nc.vector.tensor_tensor(v, si, tg, mult)
                            if u_late:
                                nc.vector.tensor_tensor(u, sf, ct[:, :], mult)
                            nc.vector.tensor_tensor(ct[:, :], u, v, add)
                            if forced:
                                nc.vector.tensor_scalar_mul(zbs[g][:, :],
                                                            ct[:, 0:1], 0.0)
                            nc.scalar.activation(tcl, ct[:, :], tanh)
                            nc.scalar.activation(so, gp[0:GP, 2 * Bg:3 * Bg],
                                                 sigm)
                            nc.vector.tensor_tensor(
                                stk_n[ROW_H1:ROW_H1 + GP, :], so, tcl, mult)
                        elif act_layout == "Yg":
                            # tanh(g) first, then sigma(ifo)
                            nc.scalar.activation(tg, gp[0:GP, 3 * Bg:4 * Bg],
                                                 tanh)
                            nc.scalar.activation(tg_t[:, 0:3 * Bg],
                                                 gp[0:GP, 0:3 * Bg], sigm)
                            nc.vector.tensor_tensor(v, si, tg, mult)
                            nc.vector.tensor_tensor(u, sf, ct[:, :], mult)
                            nc.vector.tensor_tensor(ct[:, :], u, v, add)
                            nc.scalar.activation(tcl, ct[:, :], tanh)
                            nc.vector.tensor_tensor(
                                stk_n[ROW_H1:ROW_H1 + GP, :], so, tcl, mult)
                        else:  # "Y"
                            nc.scalar.activation(tg_t[:, 0:3 * Bg],
                                                 gp[0:GP, 0:3 * Bg], sigm)
                            nc.vector.tensor_tensor(u, sf, ct[:, :], mult)
                            nc.scalar.activation(tg, gp[0:GP, 3 * Bg:4 * Bg],
                                                 tanh)
                            nc.vector.tensor_tensor(v, si, tg, mult)
                            nc.vector.tensor_tensor(ct[:, :], u, v, add)
                            nc.scalar.activation(tcl, ct[:, :], tanh)
                            nc.vector.tensor_tensor(
                                stk_n[ROW_H1:ROW_H1 + GP, :], so, tcl, mult)
    nc.compile()
    return nc


def build_core_kernel_xo(T, B, dt_c="bf16"):
    """XO: X-layout but with the two groups' sigma(o) merged into ONE
    cross-group ACT instruction (o-banks adjacent in a shared PSUM region).
    sigma(o) is off the critical chain for both groups (only needed by the
    late h-update), so the merge saves one ACT instr (+seq overhead) per
    step without touching the recurrence path.

    Shared gp region blocks (Bg cols each): [iA fA gA oA oB gB fB iB].
    Per q: A-phase emits sigma(if_A), tanh(g_A), tanh(c_A);
           B-phase emits sigma(oA|oB), ht_A, sigma(if_B) ... ht_B.
    """
    import concourse.bacc as bacc
    import concourse.mybir as mybir
    from concourse.tile import TileContext

    fp = mybir.dt.float32
    bf = mybir.dt.bfloat16
    fc = bf if dt_c == "bf16" else fp
    groups = 2
    Bg = B // groups

    nc = bacc.Bacc("TRN2", target_bir_lowering=False, debug=False)
    xT = nc.dram_tensor("xT", [T + 1, B], bf, kind="ExternalInput")
    WG = nc.dram_tensor("WG", [K_STK, 12 * MW + 1], bf, kind="ExternalInput")
    out_bt = nc.dram_tensor("out_bt", [B, T], fp, kind="ExternalOutput")

    C = min(128, T)
    assert T % C == 0 and Bg % 128 == 0

    # weight blocks built with order i,f,o,g -> indices i=0,f=1,o=2,g=3
    WB = {"i": 0, "f": 1, "o": 2, "g": 3}
    # gp column block per (group, bank)
    COL = {(0, "i"): 0, (0, "f"): 1, (0, "g"): 2, (0, "o"): 3,
           (1, "o"): 4, (1, "g"): 5, (1, "f"): 6, (1, "i"): 7}
    MMO = {0: ("i", "f", "g", "o"), 1: ("o", "f", "i", "g")}

    with TileContext(nc) as tc:
        with (
            tc.tile_pool(name="persist", bufs=1) as persist,
            tc.tile_pool(name="gpsum", bufs=1, space="PSUM") as gpsum,
            tc.tile_pool(name="opsum", bufs=1, space="PSUM") as opsum,
            tc.tile_pool(name="temps", bufs=3) as temps,
            tc.tile_pool(name="ostage", bufs=2) as ostage,
        ):
            wg = persist.tile([K_STK, 12 * MW + 1], bf)
            nc.sync.dma_start(out=wg, in_=WG[:, :])

            nchunk = Bg // 128
            gpall = gpsum.tile([GP, 8 * Bg], fp, name="gpall")
            nc.vector.memset(gpall[:, :], 0.0)
            stks, cts, pos = [], [], []
            for g in range(groups):
                pair = []
                for par in range(2):
                    stk = persist.tile([K_STK, Bg], bf, tag=f"stk{g}_{par}",
                                       name=f"stk{g}_{par}")
                    nc.vector.memset(stk[:, :], 0.0)
                    nc.sync.dma_start(out=stk[ROW_ONES:ROW_ONES + 1, :],
                                      in_=xT[0:1, g * Bg:(g + 1) * Bg])
                    pair.append(stk)
                ct = persist.tile([GP, Bg], fc, tag=f"ct{g}", name=f"ct{g}")
                nc.vector.memset(ct[:, :], 0.0)
                stks.append(pair)
                cts.append(ct)
                pos.append(opsum.tile([128, nchunk * C], fp, tag=f"po{g}",
                                      name=f"po{g}"))
            for g in range(groups):
                cols = slice(g * Bg, (g + 1) * Bg)
                for par in range(2):
                    nc.sync.dma_start(out=stks[g][par][ROW_X:ROW_X + 1, :],
                                      in_=xT[par + 1:par + 2, cols])

            add = mybir.AluOpType.add
            mult = mybir.AluOpType.mult
            tanh = mybir.ActivationFunctionType.Tanh
            sigm = mybir.ActivationFunctionType.Sigmoid

            def blk(c0, n=1):
                return gpall[0:GP, c0 * Bg:(c0 + n) * Bg]

            tcl_t = [None, None]

            for q in range(T + 2):
                for g in range(groups):
                    ct = cts[g]
                    stk = stks[g][q % 2]
                    stk_n = stks[g][(q + 1) % 2]
                    cols = slice(g * Bg, (g + 1) * Bg)
                    if q + 2 < T:
                        nc.sync.dma_start(out=stk[ROW_X:ROW_X + 1, :],
                                          in_=xT[q + 3:q + 4, cols])
                    rhs = stk[0:K_STK, :]
                    if q <= T:
                        blk0 = 4 if q == 0 else (8 if q == T else 0)
                        for bank in MMO[g]:
                            b = blk0 + WB[bank]
                            nc.tensor.matmul(
                                blk(COL[(g, bank)]),
                                wg[0:K_STK, b * MW:(b + 1) * MW],
                                rhs, start=True, stop=True)
                    # ---- B-phase: merged sigma over [oA | oB], then the
                    # delayed ht_A (needs tcl_A of this q)
                    soo = None
                    if g == 1 and q <= T:
                        soo = temps.tile([GP, 2 * Bg], bf, tag="soo",
                                         name="soo")
                        nc.scalar.activation(soo[:, :], blk(3, 2), sigm)
                        nc.vector.tensor_tensor(
                            stks[0][(q + 1) % 2][ROW_H1:ROW_H1 + GP, :],
                            soo[:, 0:Bg], tcl_t[0], mult)
                    # ---- head for step t = q-2
                    if q >= 2:
                        t = q - 2
                        tc_col = t % C
                        for k in range(nchunk):
                            nc.tensor.matmul(
                                pos[g][:, k * C + tc_col:k * C + tc_col + 1],
                                stk[64:116, k * 128:(k + 1) * 128],
                                wg[64:116, 12 * MW:12 * MW + 1],
                                start=True, stop=True)
                        if tc_col == C - 1:
                            t0 = t - (C - 1)
                            for k in range(nchunk):
                                st = ostage.tile([128, C], fp, tag=f"os{g}_{k}",
                                                 name=f"os{g}_{k}")
                                nc.vector.tensor_scalar_add(
                                    st, pos[g][:, k * C:(k + 1) * C], 0.0)
                                row0 = g * Bg + k * 128
                                nc.sync.dma_start(
                                    out=out_bt[row0:row0 + 128, t0:t0 + C],
                                    in_=st)
                    # ---- sigma(if), tanh(g), u, v, c', tanh(c)
                    if q <= T:
                        sg_t = temps.tile([GP, 2 * Bg], bf, tag=f"sg{g}",
                                          name=f"sg{g}")
                        ci = COL[(g, "i")]
                        cf = COL[(g, "f")]
                        lo = min(ci, cf)
                        nc.scalar.activation(sg_t[:, :], blk(lo, 2), sigm)
                        si = sg_t[:, 0:Bg] if ci == lo else sg_t[:, Bg:2 * Bg]
                        sf = sg_t[:, Bg:2 * Bg] if ci == lo else sg_t[:, 0:Bg]
                        u = temps.tile([GP, Bg], fc, tag=f"u{g}", name=f"u{g}")
                        v = temps.tile([GP, Bg], fc, tag=f"v{g}", name=f"v{g}")
                        tcl = temps.tile([GP, Bg], bf, tag=f"tc{g}",
                                         name=f"tc{g}")
                        nc.vector.tensor_tensor(u, sf, ct[:, :], mult)
                        tgt = temps.tile([GP, Bg], bf, tag=f"tg{g}",
                                         name=f"tg{g}")
                        nc.scalar.activation(tgt, blk(COL[(g, "g")]), tanh)
                        nc.vector.tensor_tensor(v, si, tgt, mult)
                        nc.vector.tensor_tensor(ct[:, :], u, v, add)
                        nc.scalar.activation(tcl, ct[:, :], tanh)
                        tcl_t[g] = tcl
                        # B's own ht right here (soo of this phase)
                        if g == 1:
                            nc.vector.tensor_tensor(
                                stk_n[ROW_H1:ROW_H1 + GP, :],
                                soo[:, Bg:2 * Bg], tcl, mult)
    nc.compile()
    return nc


def build_core_kernel_z(T, B, dt_c="bf16"):
    """Z-layout: 2 groups, cross-half sigma pairing.

    Per half-period (q, G): matmuls for G write into a SHARED PSUM region
    whose Bg-column blocks are laid out [fA gA oB iA fB gB oA iB] so that
      sigma instr of half (q,A) = [oB(q-1) | iA(q)]   (cols 2,3)
      tanh  instr of half (q,A) = [fA | gA]           (cols 0,1)
      sigma instr of half (q,B) = [oA(q) | iB(q)]     (cols 6,7)
      tanh  instr of half (q,B) = [fB | gB]           (cols 4,5)
    The sigma instr delivers the OTHER group's sigma(o) exactly when its
    delayed h-update (ht' = sigma_o' * tcl') is due, so the recurrence
    chain per half is just MM-i -> sigma[o'|i] -> ht' -> next MM, while
    f rides the (tanh+1) form: u=(tf+1)c, v=sigma_i*tg, c'=0.5u+v.
    States c and h stored PLAIN. 3 ACT instrs per group-step.
    """
    import concourse.bacc as bacc
    import concourse.mybir as mybir
    from concourse.tile import TileContext

    fp = mybir.dt.float32
    bf = mybir.dt.bfloat16
    fc = bf if dt_c == "bf16" else fp
    groups = 2
    Bg = B // groups

    nc = bacc.Bacc("TRN2", target_bir_lowering=False, debug=False)
    xT = nc.dram_tensor("xT", [T + 1, B], bf, kind="ExternalInput")
    WG = nc.dram_tensor("WG", [K_STK, 12 * MW + 1], bf, kind="ExternalInput")
    out_bt = nc.dram_tensor("out_bt", [B, T], fp, kind="ExternalOutput")

    C = min(128, T)
    assert T % C == 0 and Bg % 128 == 0

    # weight block index by bank (order in _build_weights: i,f,o,g)
    BI, BF_, BO, BG_ = 0, 1, 2, 3
    # gp column block per (group, bank)
    COL = {(0, BF_): 0, (0, BG_): 1, (1, BO): 2, (0, BI): 3,
           (1, BF_): 4, (1, BG_): 5, (0, BO): 6, (1, BI): 7}

    with TileContext(nc) as tc:
        with (
            tc.tile_pool(name="persist", bufs=1) as persist,
            tc.tile_pool(name="gpsum", bufs=1, space="PSUM") as gpsum,
            tc.tile_pool(name="opsum", bufs=1, space="PSUM") as opsum,
            tc.tile_pool(name="temps", bufs=3) as temps,
            tc.tile_pool(name="ostage", bufs=2) as ostage,
        ):
            wg = persist.tile([K_STK, 12 * MW + 1], bf)
            nc.sync.dma_start(out=wg, in_=WG[:, :])

            nchunk = Bg // 128
            gpall = gpsum.tile([GP, 8 * Bg], fp, name="gpall")
            nc.vector.memset(gpall[:, :], 0.0)
            stks, cts, pos = [], [], []
            for g in range(groups):
                pair = []
                for par in range(2):
                    stk = persist.tile([K_STK, Bg], bf, tag=f"stk{g}_{par}",
                                       name=f"stk{g}_{par}")
                    nc.vector.memset(stk[:, :], 0.0)
                    nc.sync.dma_start(out=stk[ROW_ONES:ROW_ONES + 1, :],
                                      in_=xT[0:1, g * Bg:(g + 1) * Bg])
                    pair.append(stk)
                ct = persist.tile([GP, Bg], fc, tag=f"ct{g}", name=f"ct{g}")
                nc.vector.memset(ct[:, :], 0.0)
                stks.append(pair)
                cts.append(ct)
                pos.append(opsum.tile([128, nchunk * C], fp, tag=f"po{g}",
                                      name=f"po{g}"))
            for g in range(groups):
                cols = slice(g * Bg, (g + 1) * Bg)
                for par in range(2):
                    nc.sync.dma_start(out=stks[g][par][ROW_X:ROW_X + 1, :],
                                      in_=xT[par + 1:par + 2, cols])

            add = mybir.AluOpType.add
            mult = mybir.AluOpType.mult
            tanh = mybir.ActivationFunctionType.Tanh
            sigm = mybir.ActivationFunctionType.Sigmoid

            def blk(c0, n=1):
                return gpall[0:GP, c0 * Bg:(c0 + n) * Bg]

            # per-group rotating temp handles for cross-half consumption
            so_t = [None, None]   # sigma_o of (g, current step)
            tcl_t = [None, None]  # tanh(c) of (g, current step)

            for q in range(T + 2):
                for g in range(groups):
                    gp_prev = 1 - g          # other group
                    ct = cts[g]
                    stk = stks[g][q % 2]
                    stk_n = stks[g][(q + 1) % 2]
                    cols = slice(g * Bg, (g + 1) * Bg)
                    if q + 2 < T:
                        nc.sync.dma_start(out=stk[ROW_X:ROW_X + 1, :],
                                          in_=xT[q + 3:q + 4, cols])
                    rhs = stk[0:K_STK, :]
                    if q <= T:
                        blk0 = 4 if q == 0 else (8 if q == T else 0)
                        for bank in (BI, BF_, BG_, BO):
                            b = blk0 + {BI: 0, BF_: 1, BO: 2, BG_: 3}[bank]
                            nc.tensor.matmul(
                                blk(COL[(g, bank)]),
                                wg[0:K_STK, b * MW:(b + 1) * MW],
                                rhs, start=True, stop=True)
                    # ---- sigma instr: [o_{G'}(prev half) | i_G(q)]
                    # col layout guarantees adjacency: (2,3) for A, (6,7) for B
                    co = COL[(gp_prev, BO)]
                    ci = COL[(g, BI)]
                    lo = min(co, ci)
                    assert ci == co + 1
                    sg_t = temps.tile([GP, 2 * Bg], bf, tag=f"sg{g}",
                                      name=f"sg{g}")
                    si = None
                    if q <= T:
                        nc.scalar.activation(sg_t[:, :], blk(lo, 2), sigm)
                        so_prev = sg_t[:, 0:Bg]
                        si = sg_t[:, Bg:2 * Bg]
                    elif q == T + 1:
                        # drain half: only sigma(o') for the final h-update
                        nc.scalar.activation(sg_t[:, 0:Bg], blk(co), sigm)
                        so_prev = sg_t[:, 0:Bg]
                    # ---- delayed h-update for the OTHER group (step q-1
                    # for g==0 half, step q for g==1 half)
                    qp = q - 1 if g == 0 else q
                    if 0 <= qp <= T and tcl_t[gp_prev] is not None:
                        nc.vector.tensor_tensor(
                            stks[gp_prev][(qp + 1) % 2][ROW_H1:ROW_H1 + GP, :],
                            so_prev, tcl_t[gp_prev], mult)
                    # ---- head for step t = q-2 of group g
                    if q >= 2:
                        t = q - 2
                        tc_col = t % C
                        for k in range(nchunk):
                            nc.tensor.matmul(
                                pos[g][:, k * C + tc_col:k * C + tc_col + 1],
                                stk[64:116, k * 128:(k + 1) * 128],
                                wg[64:116, 12 * MW:12 * MW + 1],
                                start=True, stop=True)
                        if tc_col == C - 1:
                            t0 = t - (C - 1)
                            for k in range(nchunk):
                                st = ostage.tile([128, C], fp, tag=f"os{g}_{k}",
                                                 name=f"os{g}_{k}")
                                nc.vector.tensor_scalar_add(
                                    st, pos[g][:, k * C:(k + 1) * C], 0.0)
                                row0 = g * Bg + k * 128
                                nc.sync.dma_start(
                                    out=out_bt[row0:row0 + 128, t0:t0 + C],
                                    in_=st)
                    # ---- tanh [f_G | g_G], then u, v, c', tanh-c
                    if q <= T:
                        fg_t = temps.tile([GP, 2 * Bg], bf, tag=f"fg{g}",
                                          name=f"fg{g}")
                        cf = COL[(g, BF_)]
                        nc.scalar.activation(fg_t[:, :], blk(cf, 2), tanh)
                        tf = fg_t[:, 0:Bg]
                        tg = fg_t[:, Bg:2 * Bg]
                        u = temps.tile([GP, Bg], fc, tag=f"u{g}", name=f"u{g}")
                        v = temps.tile([GP, Bg], fc, tag=f"v{g}", name=f"v{g}")
                        tcl = temps.tile([GP, Bg], bf, tag=f"tc{g}",
                                         name=f"tc{g}")
                        # u = (tf+1)*c ; v = si*tg ; c' = 0.5u + v
                        nc.vector.scalar_tensor_tensor(u, tf, 1.0, ct[:, :],
                                                       add, mult)
                        nc.vector.tensor_tensor(v, si, tg, mult)
                        nc.vector.scalar_tensor_tensor(ct[:, :], u, 0.5, v,
                                                       mult, add)
                        nc.scalar.activation(tcl, ct[:, :], tanh)
                        tcl_t[g] = tcl
    nc.compile()
    return nc


_NC_CACHE = {}


def _get_nc(T, B, groups=2, act_layout="X", dt_c="bf16"):
    key = (T, B, groups, act_layout, dt_c)
    if key not in _NC_CACHE:
        _NC_CACHE[key] = build_core_kernel(T, B, groups, act_layout, dt_c)
    return _NC_CACHE[key]


def kernel(input, W_ih1, W_hh1, b_ih1, b_hh1, W_ih2, W_hh2, b_ih2, b_hh2,
           W_lin, b_lin, _groups=2, _act_layout="X", _dt_c="bf16"):
    from concourse import bass_utils
    import ml_dtypes

    input = np.asarray(input, dtype=np.float32)
    B, T = input.shape
    Bc = B // N_CORES
    if _act_layout in ("W", "W2"):
        wkw = dict(order=("i", "f", "g", "o"), row_scale={"o": 0.5},
                   h_scale=0.5, head_scale=0.5)
    elif _act_layout == "V":
        wkw = dict(order=("f", "g", "i", "o"), row_scale={"f": 0.5})
    else:
        wkw = dict(order=("i", "f", "o", "g"))
    WG = _build_weights(np.asarray(W_ih1, np.float64), np.asarray(W_hh1, np.float64),
                        np.asarray(b_ih1, np.float64), np.asarray(b_hh1, np.float64),
                        np.asarray(W_ih2, np.float64), np.asarray(W_hh2, np.float64),
                        np.asarray(b_ih2, np.float64), np.asarray(b_hh2, np.float64),
                        np.asarray(W_lin, np.float64), np.asarray(b_lin, np.float64),
                        **wkw)
    xT = np.concatenate([np.ones((1, B), np.float32), input.T.astype(np.float32)])
    WGb = WG.astype(ml_dtypes.bfloat16)
    xTb = xT.astype(ml_dtypes.bfloat16)
    nc = _get_nc(T, Bc, _groups, _act_layout, _dt_c)
    in_maps = [
        {"xT": np.ascontiguousarray(xTb[:, c * Bc:(c + 1) * Bc]), "WG": WGb}
        for c in range(N_CORES)
    ]
    res = bass_utils.run_bass_kernel_spmd(
        nc, in_maps, core_ids=list(range(N_CORES)), trace=False)
    outs = [res.results[c]["out_bt"] for c in range(N_CORES)]
    out = np.concatenate(outs, axis=0)
    return out.astype(np.float32)
